# revision 2
# baseline (speedup 1.0000x reference)
"""Trainium2 Bass kernel for nn_CNN_Mem (CNN text encoder + cosine memory lookup).

Strategy (8 NeuronCores, SPMD):
  - Memory bank sharded along mem_size: host label-sorts mem_keys so every
    16-column block holds a single label (groups padded by duplicating a real
    key of the same label -> maxes are exact), casts to fp16, transposes to
    [300, M/8] slabs per core.
  - Each core: CNN for its 16 batch rows (embedding rows gathered host-side,
    convs as PSUM-accumulated matmuls over shifted APs, relu+bias on ACT,
    maxpool on DVE) -> feature chunks [100, 16] per kernel size. These are
    exactly the d-chunks of q^T. AllGather across the 8 cores -> lhsT
    [100, 128] per d-chunk.
  - Stream the keysT slab through the PE in [128, 512] PSUM chunks
    (3 accumulated fp16 matmuls each), segmented reduce_max (blocks of 16)
    -> per-block maxes; then one small masked-max pass over block labels
    gives per-core sim_pos/sim_neg partials (shifted by +SHIFT so empty
    positives read as 0).
  - Host combines: max over cores, un-shift, divide by feature norms (maxes
    commute with the positive per-row normalization, so the kernel works on
    unnormalized features; norms come back via a sumsq output), then
    loss = mean(relu(sim_neg - sim_pos + margin)),
    accuracy = mean(sim_pos > sim_neg)  (equivalent to the argmax form for
    distinct maxima: the nearest neighbour's label matches y iff the best
    positive beats the best negative).
"""
import numpy as np
from contextlib import ExitStack

import concourse.bass as bass
import concourse.tile as tile
from concourse import bacc, mybir
from concourse.bass_utils import run_bass_kernel_spmd

# ---- problem dims (hardcoded; harness passes matching inputs) ----
B, L = 128, 64
V, D = 25000, 300
C = 1000
KN = 100
KSIZES = (3, 4, 5)
M, KEY = 262144, 300
MARGIN = 0.1

N_CORES = 8
BPC = B // N_CORES          # batch rows per core
TOK = BPC * L               # tokens per core
DCN = 3                     # d-chunks of 100
DCW = 100                   # d-chunk width
CHUNK = 512                 # sim columns per PSUM chunk
BLK = 16                    # label-pure block width
NCH = 66                    # chunks per core
G = 6                       # chunks per DMA group
W = NCH * CHUNK             # slab columns per core (33792)
CAP = N_CORES * W           # padded memory size (270336)
NBLK = W // BLK             # blocks per core (2112)
SHIFT = 32.0                # bias added before masked max (|sims_u| <= ~8)

f32 = mybir.dt.float32
f16 = mybir.dt.float16

_CACHED_NC = None


def build():
    nc = bacc.Bacc("TRN2", target_bir_lowering=False, debug=False,
                   num_devices=N_CORES)

    kt_in = [nc.declare_dram_parameter(f"kt{c}", [DCW, W], f16, isOutput=False)
             for c in range(DCN)]
    et_in = nc.declare_dram_parameter("et", [DCN, DCW, TOK], f16,
                                      isOutput=False)
    wt_in = {k: nc.declare_dram_parameter(f"wt{k}", [DCW, k * DCN * KN], f16,
                                          isOutput=False) for k in KSIZES}
    bias_in = {k: nc.declare_dram_parameter(f"bias{k}", [DCW, 1], f32,
                                            isOutput=False) for k in KSIZES}
    y_in = nc.declare_dram_parameter("yv", [B, 1], f32, isOutput=False)
    blab_in = nc.declare_dram_parameter("blab", [1, NBLK], f32, isOutput=False)

    pos_out = nc.declare_dram_parameter("pos", [B, 1], f32, isOutput=True)
    neg_out = nc.declare_dram_parameter("neg", [B, 1], f32, isOutput=True)
    ss_out = nc.declare_dram_parameter("ss", [1, BPC], f32, isOutput=True)

    cc_in = nc.dram_tensor("cc_in", [DCW, DCN * BPC], f32)
    cc_out = nc.dram_tensor("cc_out", [N_CORES, DCW, DCN * BPC], f32,
                            addr_space="Shared")

    with tile.TileContext(nc) as tc, ExitStack() as ctx:
        singles = ctx.enter_context(tc.tile_pool(name="singles", bufs=1))
        ktp = ctx.enter_context(tc.tile_pool(name="ktp", bufs=4))
        work = ctx.enter_context(tc.tile_pool(name="work", bufs=1))

        # ---------------- CNN phase ----------------
        et = []
        for dc in range(DCN):
            t = singles.tile([DCW, TOK], f16, name=f"et{dc}", tag=f"et{dc}")
            nc.sync.dma_start(out=t, in_=et_in[dc, :, :])
            et.append(t)
        wt = {}
        bia = {}
        for k in KSIZES:
            wt[k] = singles.tile([DCW, k * DCN * KN], f16, name=f"wt{k}", tag=f"wt{k}")
            nc.sync.dma_start(out=wt[k], in_=wt_in[k][:, :])
            bia[k] = singles.tile([DCW, 1], f32, name=f"bias{k}", tag=f"bias{k}")
            nc.sync.dma_start(out=bia[k], in_=bias_in[k][:, :])

        feats = {}  # per kernel size: [100, BPC] f32 (this IS qT d-chunk)
        with tc.tile_pool(name="cnnps", bufs=2, space="PSUM") as cnnps, \
             tc.tile_pool(name="cnnsb", bufs=2) as cnnsb:
            for k in KSIZES:
                lout = L - k + 1
                half = BPC // 2
                fk = singles.tile([DCW, BPC], f32, name=f"feats{k}", tag=f"feats{k}")
                feats[k] = fk
                for h in range(2):
                    ps = cnnps.tile([DCW, half * lout], f32, tag="cnnpsum")
                    first = True
                    for t in range(k):
                        for dc in range(DCN):
                            rhs = et[dc].rearrange(
                                "p (b l) -> p b l", l=L)[:, h * half:(h + 1) * half,
                                                         t:t + lout]
                            nc.tensor.matmul(
                                ps[:],
                                wt[k][:, (t * DCN + dc) * KN:(t * DCN + dc + 1) * KN],
                                rhs,
                                start=first, stop=(t == k - 1 and dc == DCN - 1))
                            first = False
                    # bias + relu (ACT), then maxpool over positions (DVE)
                    rk = cnnsb.tile([DCW, half * lout], f32, tag="relu")
                    nc.scalar.activation(rk[:], ps[:],
                                         mybir.ActivationFunctionType.Relu,
                                         bias=bia[k][:], scale=1.0)
                    nc.vector.tensor_reduce(
                        out=fk[:, h * half:(h + 1) * half],
                        in_=rk.rearrange("p (b l) -> p b l", l=lout),
                        axis=mybir.AxisListType.X, op=mybir.AluOpType.max)

            # sumsq of features per local batch row: ss[1, BPC]
            ones = singles.tile([DCW, 1], f32, tag="ones")
            nc.vector.memset(ones, 1.0)
            ssps = cnnps.tile([1, BPC], f32, tag="ssps")
            for i, k in enumerate(KSIZES):
                sq = cnnsb.tile([DCW, BPC], f32, tag="sq")
                nc.vector.tensor_mul(sq[:], feats[k][:], feats[k][:])
                nc.tensor.matmul(ssps[:], ones[:], sq[:],
                                 start=(i == 0), stop=(i == len(KSIZES) - 1))
            ss_sb = singles.tile([1, BPC], f32, tag="ss_sb")
            nc.vector.tensor_copy(ss_sb[:], ssps[:])
            nc.sync.dma_start(out=ss_out[:, :], in_=ss_sb[:])

        # ---------------- AllGather features ----------------
        fall = singles.tile([DCW, DCN * BPC], f32, tag="fall")
        for i, k in enumerate(KSIZES):
            nc.vector.tensor_copy(fall[:, i * BPC:(i + 1) * BPC], feats[k][:])
        nc.sync.dma_start(out=cc_in[:, :], in_=fall[:])
        nc.gpsimd.collective_compute(
            "AllGather", mybir.AluOpType.bypass,
            replica_groups=[list(range(N_CORES))],
            ins=[cc_in[:, :]], outs=[cc_out[:, :, :]])

        qt = []
        for dc in range(DCN):
            q32 = singles.tile([DCW, N_CORES, BPC], f32, name=f"q32_{dc}", tag=f"q32_{dc}")
            src = bass.AP(tensor=cc_out.ap().tensor,
                          offset=dc * BPC,
                          ap=[[DCN * BPC, DCW], [DCW * DCN * BPC, N_CORES],
                              [1, BPC]])
            nc.sync.dma_start(out=q32, in_=src)
            q16 = singles.tile([DCW, N_CORES * BPC], f16, name=f"q16_{dc}", tag=f"q16_{dc}")
            nc.vector.tensor_copy(q16[:], q32.rearrange("p a b -> p (a b)"))
            qt.append(q16)

        # ---------------- memory stream ----------------
        bmall = work.tile([B, NBLK], f32, tag="bmall")
        with tc.tile_pool(name="simps", bufs=8, space="PSUM") as simps:
            ngroups = (NCH + G - 1) // G
            for g in range(ngroups):
                j0 = g * G
                gw = min(G, NCH - j0) * CHUNK
                kt = []
                for dc in range(DCN):
                    t = ktp.tile([DCW, G * CHUNK], f16, name=f"ktt{dc}", tag=f"kt{dc}")
                    nc.sync.dma_start(
                        out=t[:, :gw], in_=kt_in[dc][:, j0 * CHUNK:j0 * CHUNK + gw])
                    kt.append(t)
                pss = []
                for j in range(gw // CHUNK):
                    pss.append(simps.tile([B, CHUNK], f32, name="simpsum", tag="simpsum"))
                for dc in range(DCN):
                    for j in range(gw // CHUNK):
                        nc.tensor.matmul(
                            pss[j][:], qt[dc][:],
                            kt[dc][:, j * CHUNK:(j + 1) * CHUNK],
                            start=(dc == 0), stop=(dc == DCN - 1))
                for j in range(gw // CHUNK):
                    nc.vector.tensor_reduce(
                        out=bmall[:, (j0 + j) * (CHUNK // BLK):
                                  (j0 + j + 1) * (CHUNK // BLK)],
                        in_=pss[j].rearrange("p (nb blk) -> p nb blk", blk=BLK),
                        axis=mybir.AxisListType.X, op=mybir.AluOpType.max)

        # ---------------- masked maxes over block labels ----------------
        blab_b = work.tile([B, NBLK], f32, tag="blab_b")
        nc.sync.dma_start(out=blab_b, in_=bass.AP(
            tensor=blab_in.ap().tensor, offset=0, ap=[[0, B], [1, NBLK]]))
        y0 = singles.tile([B, 1], f32, tag="y0")
        nc.sync.dma_start(out=y0, in_=y_in[:, :])
        yv = singles.tile([B, 1], f32, tag="yv")
        nc.vector.tensor_copy(yv[:], y0[:])

        eq = work.tile([B, NBLK], f32, tag="eq")
        nc.vector.tensor_scalar(out=eq[:], in0=blab_b[:], scalar1=yv[:],
                                scalar2=None, op0=mybir.AluOpType.is_equal)
        posm = work.tile([B, NBLK], f32, tag="posm")
        nc.vector.scalar_tensor_tensor(
            out=posm[:], in0=bmall[:], scalar=SHIFT, in1=eq[:],
            op0=mybir.AluOpType.add, op1=mybir.AluOpType.mult)
        neq = work.tile([B, NBLK], f32, tag="neq")
        nc.vector.tensor_scalar(out=neq[:], in0=blab_b[:], scalar1=yv[:],
                                scalar2=None, op0=mybir.AluOpType.not_equal)
        negm = work.tile([B, NBLK], f32, tag="negm")
        nc.vector.scalar_tensor_tensor(
            out=negm[:], in0=bmall[:], scalar=SHIFT, in1=neq[:],
            op0=mybir.AluOpType.add, op1=mybir.AluOpType.mult)

        pos = singles.tile([B, 1], f32, tag="pos")
        nc.vector.tensor_reduce(out=pos[:], in_=posm[:],
                                axis=mybir.AxisListType.X,
                                op=mybir.AluOpType.max)
        nc.sync.dma_start(out=pos_out[:, :], in_=pos[:])
        neg = singles.tile([B, 1], f32, tag="neg")
        nc.vector.tensor_reduce(out=neg[:], in_=negm[:],
                                axis=mybir.AxisListType.X,
                                op=mybir.AluOpType.max)
        nc.sync.dma_start(out=neg_out[:, :], in_=neg[:])

    nc.compile()
    return nc


def _prep(x, y, embed, conv_w3, conv_b3, conv_w4, conv_b4, conv_w5, conv_b5,
          mem_keys, mem_values):
    """Host-side sharding/packing. Returns per-core input maps + combine data."""
    x = np.asarray(x)
    y64 = np.asarray(y).astype(np.int64)
    mv = np.asarray(mem_values).astype(np.int64)
    mk = np.asarray(mem_keys, dtype=np.float32)

    # --- label-sorted, block-pure padded permutation of the memory bank ---
    order = np.argsort(mv, kind="stable")
    cnt = np.bincount(mv, minlength=C)
    assert cnt.min() > 0, "kernel assumes every class present in memory"
    starts = np.zeros(C + 1, np.int64)
    starts[1:] = np.cumsum(cnt)
    parts = []
    for c in range(C):
        g = order[starts[c]:starts[c + 1]]
        padn = (-len(g)) % BLK
        if padn:
            g = np.concatenate([g, np.repeat(g[0], padn)])
        parts.append(g)
    perm = np.concatenate(parts)
    assert len(perm) <= CAP, f"padded size {len(perm)} exceeds CAP {CAP}"
    perm = np.concatenate([perm, np.repeat(perm[0], CAP - len(perm))])
    labP = mv[perm]
    blab = labP[::BLK].astype(np.float32)          # [CAP // BLK]
    keysT = np.ascontiguousarray(mk[perm].astype(np.float16).T)  # [300, CAP]

    # --- embedding lookup (host gather; device gets ready eT slabs) ---
    emb16 = np.asarray(embed, dtype=np.float32).astype(np.float16)
    e = emb16[x]                                    # [B, L, 300]
    # eT[dc, p, b*L + l] = e[b, l, dc*100 + p]
    eT = np.ascontiguousarray(
        e.reshape(B, L, DCN, DCW).transpose(2, 3, 0, 1).reshape(DCN, DCW, B * L))

    # --- conv weights: wt[k][p, (t*3+dc)*KN + kn] = w_k[kn, dc*100+p, t] ---
    wts, biases = {}, {}
    for k, w_, b_ in ((3, conv_w3, conv_b3), (4, conv_w4, conv_b4),
                      (5, conv_w5, conv_b5)):
        w_ = np.asarray(w_, dtype=np.float32)       # [KN, D, k]
        a = w_.reshape(KN, DCN, DCW, k).transpose(3, 1, 2, 0)  # [t, dc, p, kn]
        wts[k] = np.ascontiguousarray(
            a.transpose(2, 0, 1, 3).reshape(DCW, k * DCN * KN)).astype(np.float16)
        biases[k] = np.asarray(b_, dtype=np.float32).reshape(DCW, 1)

    yv = y64.astype(np.float32).reshape(B, 1)

    in_maps = []
    for c in range(N_CORES):
        m = {
            "et": np.ascontiguousarray(
                eT.reshape(DCN, DCW, B, L)[:, :, c * BPC:(c + 1) * BPC, :]
                .reshape(DCN, DCW, TOK)),
            "yv": yv,
            "blab": np.ascontiguousarray(
                blab[c * NBLK:(c + 1) * NBLK]).reshape(1, NBLK),
        }
        for dc in range(DCN):
            m[f"kt{dc}"] = np.ascontiguousarray(
                keysT[dc * DCW:(dc + 1) * DCW, c * W:(c + 1) * W])
        for k in KSIZES:
            m[f"wt{k}"] = wts[k]
            m[f"bias{k}"] = biases[k]
        in_maps.append(m)
    return in_maps, y64


def _combine(results, y64):
    pos = np.max([r["pos"].reshape(B) for r in results], axis=0) - SHIFT
    neg = np.max([r["neg"].reshape(B) for r in results], axis=0) - SHIFT
    ss = np.concatenate([r["ss"].reshape(BPC) for r in results])  # [B]
    rn = 1.0 / np.maximum(np.sqrt(ss), 1e-12)
    sp = pos * rn
    sn = neg * rn
    loss = np.float32(np.mean(np.maximum(sn - sp + MARGIN, 0.0)))
    acc = np.float32(np.mean((sp > sn).astype(np.float32)))
    return loss, acc


def kernel(**inputs):
    global _CACHED_NC
    in_maps, y64 = _prep(**inputs)
    if _CACHED_NC is None:
        _CACHED_NC = build()
    res = run_bass_kernel_spmd(_CACHED_NC, in_maps,
                               core_ids=list(range(N_CORES)))
    return _combine(res.results, y64)
